# revision 26
# baseline (speedup 1.0000x reference)
import sys
import numpy as np

sys.path.insert(0, "/opt/trn_rl_repo")

import concourse.bass as bass  # noqa: E402
import concourse.tile as tile  # noqa: E402
from concourse import bacc, mybir  # noqa: E402
from concourse.ap import AP  # noqa: E402
from concourse.bass_utils import run_bass_kernel_spmd  # noqa: E402
import ml_dtypes  # noqa: E402

BF16 = mybir.dt.bfloat16
F32 = mybir.dt.float32
FP8 = mybir.dt.float8e4

S = 48                 # corner cube side (abs cube coords [22, 70))
C0 = 22                # corner offset in cube coords
SS = S * S             # 2304
CUBE48 = S * SS        # 110592
V1 = 46                # conv1 computed outputs per axis (abs [22, 68))
P2 = 26                # conv2 padded input side (H2 abs [8, 34))
P2S = P2 * P2          # 676
H2C = 23               # H2 corner side (abs [11, 34))
V2 = 24                # conv2 computed outputs per axis (abs [8, 32))
H3S = 16

_CACHE = {}


def _build():
    nc = bacc.Bacc("TRN2", target_bir_lowering=False, debug=False, num_devices=8)
    xin_d = nc.dram_tensor("xin", [S, 6 * SS], BF16, kind="ExternalInput")
    gt_d = nc.dram_tensor("gt", [S, 6 * S], BF16, kind="ExternalInput")
    w1a_d = nc.dram_tensor("w1a", [108, 32], BF16, kind="ExternalInput")
    w1b_d = nc.dram_tensor("w1b", [54, 32], BF16, kind="ExternalInput")
    w2_d = nc.dram_tensor("w2", [96, 576], BF16, kind="ExternalInput")
    w3_d = nc.dram_tensor("w3", [64, 27 * 128], BF16, kind="ExternalInput")
    w4_d = nc.dram_tensor("w4", [128, 27 * 256], BF16, kind="ExternalInput")
    f1_d = nc.dram_tensor("f1", [128, 16 * 1024], BF16, kind="ExternalInput")
    f2_d = nc.dram_tensor("f2", [128, 8 * 29], BF16, kind="ExternalInput")
    b1_d = nc.dram_tensor("b1", [32, 1], F32, kind="ExternalInput")
    b2_d = nc.dram_tensor("b2", [64, 1], F32, kind="ExternalInput")
    bg2_d = nc.dram_tensor("bg2", [32, 1], F32, kind="ExternalInput")
    bg3_d = nc.dram_tensor("bg3", [64, 1], F32, kind="ExternalInput")
    b3_d = nc.dram_tensor("b3", [128, 1], F32, kind="ExternalInput")
    b4_d = nc.dram_tensor("b4", [128, 2], F32, kind="ExternalInput")
    fb1_d = nc.dram_tensor("fb1", [128, 8], F32, kind="ExternalInput")
    fb2_d = nc.dram_tensor("fb2", [29, 1], F32, kind="ExternalInput")
    y_d = nc.dram_tensor("y", [29], F32, kind="ExternalOutput")
    cube_d = nc.dram_tensor("cube", [6 * CUBE48 + 4096], FP8)

    Relu = mybir.ActivationFunctionType.Relu
    Copy = mybir.ActivationFunctionType.Copy
    amax = mybir.AluOpType.max
    aadd = mybir.AluOpType.add

    with tile.TileContext(nc, pool_alloc_mode="queue") as tc:
        with (
            tc.tile_pool(name="const", bufs=1) as constp,
        ):
            gt = constp.tile([S, 6 * S], BF16)
            w1a = constp.tile([108, 32], BF16)
            w1b = constp.tile([54, 32], BF16)
            w2 = constp.tile([96, 576], BF16)
            b1 = constp.tile([32, 1], F32)
            b2 = constp.tile([64, 1], F32)
            bg2 = constp.tile([32, 1], F32)
            bg3 = constp.tile([64, 1], F32)
            b3 = constp.tile([128, 1], F32)
            b4 = constp.tile([128, 2], F32)
            fb1 = constp.tile([128, 8], F32)
            fb2 = constp.tile([29, 1], F32)

            # T1 chunk z-ranges for conv1: output z' in [z0, z1), needs cube
            # slices [z0, z1+2)
            zchunks = [(0, 4), (4, 12), (12, 20), (20, 28), (28, 36), (36, 44), (44, 46)]

            # pools in LIFO lifetime order: h3 (to conv3), T2 (to conv2),
            # T1 (to conv1), xin (blur only)
            h3pool = tc.alloc_tile_pool(name="h3pool", bufs=1)
            H3 = h3pool.tile([64, H3S * H3S * H3S], BF16)
            t2pool = tc.alloc_tile_pool(name="t2pool", bufs=1)
            T2 = t2pool.tile([96, P2 * P2S], BF16)
            t1pool = tc.alloc_tile_pool(name="t1pool", bufs=2)
            MAXCOLS = 10 * SS
            T1 = [t1pool.tile([108, MAXCOLS], FP8, tag="t1", name=f"T1_{i}")
                  for i in range(2)]

            # ---------------- blur (48^3 corner, separable) ----------------
            xinp = tc.alloc_tile_pool(name="xinp", bufs=1)
            xin = xinp.tile([S, 6 * SS], BF16)
            # critical-path loads first, late-needed consts after
            nc.sync.dma_start(gt[:], gt_d[:])
            nc.sync.dma_start(xin[:], xin_d[:])
            for t_, d_ in [(w1a, w1a_d), (w1b, w1b_d), (bg2, bg2_d),
                           (b1, b1_d), (w2, w2_d), (b2, b2_d), (bg3, bg3_d),
                           (b3, b3_d), (b4, b4_d), (fb1, fb1_d), (fb2, fb2_d)]:
                nc.sync.dma_start(t_[:], d_[:])
            xr = xin[:].rearrange("p (e j k) -> p e j k", e=6, j=S, k=S)

            with (
                tc.tile_pool(name="t12", bufs=6) as t12p,
                tc.tile_pool(name="bpsA", bufs=5, space="PSUM") as bpsA,
                tc.tile_pool(name="bpsC", bufs=2, space="PSUM") as bpsC,
                tc.tile_pool(name="cstp", bufs=2) as cstp,
            ):
                # k/a groups for psum staging in stages A and B
                groups = [(0, 10), (10, 10), (20, 10), (30, 10), (40, 8)]
                t1s, t2s = {}, {}

                def stageA(e):
                    ge = gt[:, e * S:(e + 1) * S]
                    t1 = t12p.tile([S, SS], BF16, tag="t", name=f"t1_{e}")
                    t1s[e] = t1
                    for gi, (k0, nk) in enumerate(groups):
                        ps = bpsA.tile([S, 480], F32, tag="ps")
                        for s in range(nk):
                            k = k0 + s
                            nc.tensor.matmul(ps[:, s * S:(s + 1) * S],
                                             xr[:, e, :, k], ge)
                        if gi % 2 == 0:
                            nc.scalar.activation(
                                t1[:, k0 * S:(k0 + nk) * S], ps[:, :nk * S], Copy)
                        else:
                            nc.vector.tensor_copy(
                                t1[:, k0 * S:(k0 + nk) * S], ps[:, :nk * S])

                def stageB(e):
                    ge = gt[:, e * S:(e + 1) * S]
                    t1r = t1s[e][:].rearrange("p (k a) -> p k a", k=S)
                    t2 = t12p.tile([S, SS], BF16, tag="t", name=f"t2_{e}")
                    t2s[e] = t2
                    for gi, (a0, na) in enumerate(groups):
                        ps = bpsA.tile([S, 480], F32, tag="ps")
                        for s in range(na):
                            a = a0 + s
                            nc.tensor.matmul(ps[:, s * S:(s + 1) * S],
                                             t1r[:, :, a], ge)
                        if gi % 2 == 0:
                            nc.scalar.activation(
                                t2[:, a0 * S:(a0 + na) * S], ps[:, :na * S], Copy)
                        else:
                            nc.vector.tensor_copy(
                                t2[:, a0 * S:(a0 + na) * S], ps[:, :na * S])

                def stageC(e):
                    # contract k -> cube chunks [(a,p) 128-chunks, q]
                    ge = gt[:, e * S:(e + 1) * S]
                    t2 = t2s[e]
                    cst = cstp.tile([128, 18 * S], FP8, tag="cst", name=f"cst_{e}")
                    for gi, (c0, ncn) in enumerate([(0, 10), (10, 8)]):
                        ps = bpsC.tile([128, 480], F32, tag="psc")
                        for s in range(ncn):
                            c = c0 + s
                            nc.tensor.matmul(ps[:, s * S:(s + 1) * S],
                                             t2[:, c * 128:(c + 1) * 128], ge)
                        if gi % 2 == 0:
                            nc.scalar.activation(
                                cst[:, c0 * S:(c0 + ncn) * S], ps[:, :ncn * S], Copy)
                        else:
                            nc.vector.tensor_copy(
                                cst[:, c0 * S:(c0 + ncn) * S], ps[:, :ncn * S])
                    cr = cst[:].rearrange("p (c q) -> p c q", c=18)
                    dst = AP(cube_d, e * CUBE48, [[S, 128], [128 * S, 18], [1, S]])
                    nc.sync.dma_start(dst, cr[:])
                    # T1 chunk 0 rows for this element (overlap with blur)
                    # rows [0:54) hold the dz'=1 slices so mm2's rhs starts at
                    # partition 0 (hw requires base partition 0/32/64)
                    z0, z1 = zchunks[0]
                    cols = (z1 + 2 - z0) * SS
                    for dzp in range(2):
                        rb = (1 - dzp) * 54
                        src = AP(cube_d, e * CUBE48 + dzp * SS + z0 * SS,
                                 [[S, 3], [1, 3], [1, cols]])
                        nc.sync.dma_start(
                            T1[0][rb + e * 9:rb + (e + 1) * 9, :cols], src)

                # software-pipelined across e: A(e+1)/B(e) overlap copy drains
                # so the PE stays continuously fed (p-state ramp)
                for step in range(8):
                    if step < 6:
                        stageA(step)
                    if 1 <= step <= 6:
                        stageB(step - 1)
                    if step >= 2:
                        stageC(step - 2)
            xinp.release()

            # ---------------- conv1 (+pool) ----------------
            # T2 holds conv2's z-replica tall tile; rows [0:32] double as H2P
            # (padded H2: bg2 halo + pooled conv1 corner).
            H2P = T2[0:32, :]

            # background fills (independent of conv1 compute)
            # H2P halo: z-planes [0,3), then y<3 rows for z>=3, then x<3 cols
            nc.gpsimd.memset(H2P[:, 0:3 * P2S], 0.0)
            hz = H2P[:, 3 * P2S:].rearrange("p (z y x) -> p z y x", z=P2 - 3, y=P2)
            nc.gpsimd.memset(hz[:, :, 0:3, :], 0.0)
            nc.gpsimd.memset(hz[:, :, 3:, 0:3], 0.0)
            nc.scalar.activation(H2P[:, 0:3 * P2S], H2P[:, 0:3 * P2S],
                                 Relu, bias=bg2[:])
            nc.scalar.activation(hz[:, :, 0:3, :], hz[:, :, 0:3, :],
                                 Relu, bias=bg2[:])
            nc.scalar.activation(hz[:, :, 3:, 0:3], hz[:, :, 3:, 0:3],
                                 Relu, bias=bg2[:])
            # H3 background fill
            nc.gpsimd.memset(H3[:], 0.0)
            nc.scalar.activation(H3[:], H3[:], Relu, bias=bg3[:])

            h2cr = H2P[:].rearrange("p (z y x) -> p z y x", z=P2, y=P2)

            with (
                tc.tile_pool(name="c1ps", bufs=4, space="PSUM") as c1ps,
                tc.tile_pool(name="c1xt", bufs=3) as c1xt,
                tc.tile_pool(name="c1yt", bufs=3) as c1yt,
            ):
                yts = {}
                copied_half1 = False
                for ci, (z0, z1) in enumerate(zchunks):
                    cols = (z1 + 2 - z0) * SS
                    T1c = T1[ci % 2]
                    if ci > 0:
                        for dzp in range(2):
                            rb = (1 - dzp) * 54
                            for e in range(6):
                                src = AP(cube_d,
                                         e * CUBE48 + dzp * SS + z0 * SS,
                                         [[S, 3], [1, 3], [1, cols]])
                                nc.sync.dma_start(
                                    T1c[rb + e * 9:rb + (e + 1) * 9, :cols],
                                    src)
                    for zp in range(z0, z1):
                        zl = (zp - z0) * SS
                        # three y-thirds: psum [32,768] = 2 banks, 4-deep
                        for hi, (y0, ny) in enumerate([(0, 16), (16, 16), (32, 14)]):
                            ncols = ny * S
                            base = zl + y0 * S
                            ps = c1ps.tile([32, 768], F32, tag="ps")
                            for co in range(0, ncols, 512):
                                cw = min(512, ncols - co)
                                nc.tensor.matmul(
                                    ps[:, co:co + cw],
                                    w1a[:], T1c[0:108, base + co:base + co + cw],
                                    start=True, stop=False)
                                nc.tensor.matmul(
                                    ps[:, co:co + cw],
                                    w1b[:], T1c[0:54, base + co + SS:base + co + SS + cw],
                                    start=False, stop=True)
                            # x-pair max: Act drains odd cols to SBUF, DVE
                            # maxes psum-even against sbuf-odd (walrus only
                            # allows one PSUM operand per DVE op)
                            pp = ps[:, 0:ncols].rearrange(
                                "p (y x) -> p y x", y=ny)[:, :, 0:46].rearrange(
                                "p y (xp two) -> p y xp two", two=2)
                            xo = c1xt.tile([32, 368], BF16, tag="xo")
                            xor_ = xo[:, 0:ny * 23].rearrange(
                                "p (y x) -> p y x", y=ny)
                            nc.scalar.activation(xor_[:], pp[:, :, :, 1], Copy)
                            xt = c1xt.tile([32, 368], BF16, tag="xt")
                            xtr = xt[:, 0:ny * 23].rearrange(
                                "p (y x) -> p y x", y=ny)
                            nc.vector.tensor_tensor(
                                xtr[:], pp[:, :, :, 0], xor_[:], amax)
                            # y-pair max into yt slice (pair ranges per third)
                            if hi == 0:
                                yt = c1yt.tile([32, 529], BF16, tag="yt")
                                yts[zp] = yt
                            else:
                                yt = yts[zp]
                            nyp = ny // 2
                            ytr = yt[:, y0 // 2 * 23:(y0 // 2 + nyp) * 23].rearrange(
                                "p (y x) -> p y x", y=nyp)
                            x2 = xt[:, 0:ny * 23].rearrange(
                                "p (y two x) -> p y two x", y=nyp, two=2)
                            nc.vector.tensor_tensor(
                                ytr[:], x2[:, :, 0, :], x2[:, :, 1, :], amax)
                        # z-pair: finish H2 corner slice m = zp//2
                        if zp % 2 == 1:
                            m = zp // 2
                            dst = h2cr[:, 3 + m, 3:, 3:]
                            nc.vector.tensor_tensor(
                                dst, yts[zp - 1][:], yts[zp][:], amax)
                            nc.scalar.activation(dst, dst, Relu, bias=b1[:])
                            del yts[zp - 1], yts[zp]
                        # mid-conv1: copy first z-half of T2 lanes (needs
                        # H2P cols z < 13, i.e. corner m <= 9 plus halo)
                        if zp == 19 and not copied_half1:
                            copied_half1 = True
                            nch = 13 * P2S
                            for lz in (1, 2):
                                nc.sync.dma_start(
                                    T2[32 * lz:32 * (lz + 1), 0:nch - lz * P2S],
                                    T2[0:32, lz * P2S:nch])
                # remaining T2 lane cols
                for lz in (1, 2):
                    nch = 13 * P2S
                    rest = P2 * P2S - lz * P2S
                    nc.sync.dma_start(
                        T2[32 * lz:32 * (lz + 1), nch - lz * P2S:rest],
                        T2[0:32, nch:P2 * P2S])

            t1pool.release()

            # ---------------- conv2 (+pool) ----------------
            h3r16 = H3[:].rearrange("p (z y x) -> p z y x", z=H3S, y=H3S)
            with (
                tc.tile_pool(name="c2ps", bufs=2, space="PSUM") as c2ps,
                tc.tile_pool(name="c2tmp", bufs=6) as c2tmp,
            ):
                y2ts = {}
                for zp in range(V2):
                    base = zp * P2S
                    ps = c2ps.tile([64, 624], F32, tag="ps")
                    for co, cw in [(0, 512), (512, 112)]:
                        for t in range(9):
                            dy, dx = t // 3, t % 3
                            off = base + dy * P2 + dx + co
                            nc.tensor.matmul(
                                ps[:, co:co + cw],
                                w2[:, t * 64:(t + 1) * 64],
                                T2[0:96, off:off + cw],
                                start=(t == 0), stop=(t == 8))
                    # pool: x-pair then y-pair (valid 24x24 of 26-stride)
                    pp = ps[:].rearrange("p (y x) -> p y x", y=24)[
                        :, :, 0:24].rearrange("p y (xp two) -> p y xp two", two=2)
                    xo = c2tmp.tile([64, 288], BF16, tag="xo")
                    xor_ = xo[:].rearrange("p (y x) -> p y x", y=24)
                    nc.scalar.activation(xor_[:], pp[:, :, :, 1], Copy)
                    xt = c2tmp.tile([64, 288], BF16, tag="xt")
                    xtr = xt[:].rearrange("p (y x) -> p y x", y=24)
                    nc.vector.tensor_tensor(
                        xtr[:], pp[:, :, :, 0], xor_[:], amax)
                    yt = c2tmp.tile([64, 144], BF16, tag="yt")
                    ytr = yt[:].rearrange("p (y x) -> p y x", y=12)
                    x2 = xt[:].rearrange("p (y two x) -> p y two x", y=12, two=2)
                    nc.vector.tensor_tensor(
                        ytr[:], x2[:, :, 0, :], x2[:, :, 1, :], amax)
                    y2ts[zp] = yt
                    if zp % 2 == 1:
                        m = zp // 2
                        dst = h3r16[:, 4 + m, 4:, 4:]
                        nc.vector.tensor_tensor(
                            dst, y2ts[zp - 1][:], y2ts[zp][:], amax)
                        nc.scalar.activation(dst, dst, Relu, bias=b2[:])
                        del y2ts[zp - 1], y2ts[zp]
            t2pool.release()
            # conv3/conv4/fc weights (loads overlap conv3 compute)
            fcp = tc.alloc_tile_pool(name="fcp", bufs=1)
            w3 = fcp.tile([64, 27 * 128], BF16)
            nc.sync.dma_start(w3[:], w3_d[:])
            w4 = fcp.tile([128, 27 * 256], BF16)
            nc.sync.dma_start(w4[:], w4_d[:])
            f1 = fcp.tile([128, 16 * 1024], BF16)
            nc.sync.dma_start(f1[:], f1_d[:])
            f2 = fcp.tile([128, 8 * 29], BF16)
            nc.sync.dma_start(f2[:], f2_d[:])

            h4p = tc.alloc_tile_pool(name="h4p", bufs=1)
            H4 = h4p.tile([128, 343], BF16)
            # ---------------- conv3 (baseline) ----------------
            with (
                tc.tile_pool(name="c3ps", bufs=8, space="PSUM") as c3ps,
                tc.tile_pool(name="c3tmp", bufs=16) as c3tmp,
            ):
                h3r = H3[:].rearrange("p (z y x) -> p z y x", z=16, y=16)
                zts = {}
                for half in range(2):
                    pss = []
                    for zi7 in range(7):
                        pszz = c3ps.tile([128, 196], F32, tag="ps")
                        pss.append(pszz)
                    for t in range(27):
                        dz, dy, dx = t // 9, (t // 3) % 3, t % 3
                        for zi in range(7):
                            z = half * 7 + zi
                            rhs = h3r[:, z + dz, dy:dy + 14, dx:dx + 14]
                            nc.tensor.matmul(pss[zi][:], w3[:, t * 128:(t + 1) * 128],
                                             rhs, start=(t == 0), stop=(t == 26))
                    for zi in range(7):
                        z = half * 7 + zi
                        ps = pss[zi]
                        pr = ps[:].rearrange("p (y xp two) -> p y xp two", y=14, two=2)
                        xt = c3tmp.tile([128, 98], F32, tag="xt")
                        xtr = xt[:].rearrange("p (y x) -> p y x", y=14)
                        nc.vector.tensor_reduce(xtr[:], pr[:], mybir.AxisListType.X, amax)
                        yt = c3tmp.tile([128, 49], F32, tag="yt")
                        ytr = yt[:].rearrange("p (y x) -> p y x", y=7)
                        xr2 = xt[:].rearrange("p (yp two x) -> p yp two x", yp=7, two=2)
                        nc.vector.tensor_tensor(ytr[:], xr2[:, :, 0, :], xr2[:, :, 1, :], amax)
                        zts[z] = yt
                for zq in range(7):
                    zt = c3tmp.tile([128, 49], F32, tag="zt")
                    nc.vector.tensor_tensor(zt[:], zts[2 * zq][:], zts[2 * zq + 1][:], amax)
                    nc.scalar.activation(H4[:, zq * 49:(zq + 1) * 49], zt[:],
                                         Relu, bias=b3[:])

            # ---------------- conv4 + fc (baseline) ----------------
            with (
                tc.tile_pool(name="c4ps", bufs=2, space="PSUM") as c4ps,
                tc.tile_pool(name="c4tmp", bufs=8) as c4tmp,
            ):
                h4r = H4[:].rearrange("p (z y x) -> p z y x", z=7, y=7)
                v = c4tmp.tile([128, 16], BF16, tag="v")
                for mt in range(2):
                    ps = c4ps.tile([128, 125], F32, tag="ps")
                    for t in range(27):
                        dz, dy, dx = t // 9, (t // 3) % 3, t % 3
                        rhs = h4r[:, dz:dz + 5, dy:dy + 5, dx:dx + 5]
                        nc.tensor.matmul(ps[:], w4[:, t * 256 + mt * 128:t * 256 + (mt + 1) * 128],
                                         rhs, start=(t == 0), stop=(t == 26))
                    pr0 = ps[:].rearrange("p (z y x) -> p z y x", z=5, y=5)
                    pr = pr0[:, :, :, 0:4].rearrange("p z y (xp two) -> p (z y) xp two", two=2)
                    xt = c4tmp.tile([128, 50], F32, tag="xt")
                    xtr = xt[:].rearrange("p (zy x) -> p zy x", x=2)
                    nc.vector.tensor_reduce(xtr[:], pr[:], mybir.AxisListType.X, amax)
                    x20 = xt[:].rearrange("p (z y x) -> p z y x", z=5, y=5)
                    x2 = x20[:, :, 0:4, :].rearrange("p z (yp two) x -> p z yp two x", two=2)
                    yt = c4tmp.tile([128, 20], F32, tag="yt")
                    ytr = yt[:].rearrange("p (z y x) -> p z y x", z=5, y=2)
                    nc.vector.tensor_tensor(ytr[:], x2[:, :, :, 0, :], x2[:, :, :, 1, :], amax)
                    y2r0 = yt[:].rearrange("p (z yx) -> p z yx", z=5)
                    y2r = y2r0[:, 0:4, :].rearrange("p (zp two) yx -> p zp two yx", two=2)
                    zt = c4tmp.tile([128, 8], F32, tag="zt")
                    ztr = zt[:].rearrange("p (z yx) -> p z yx", z=2)
                    nc.vector.tensor_tensor(ztr[:], y2r[:, :, 0, :], y2r[:, :, 1, :], amax)
                    nc.scalar.activation(v[:, mt * 8:(mt + 1) * 8], zt[:],
                                         Relu, bias=b4[:, mt:mt + 1])
                # fc1
                ps5 = c4ps.tile([128, 8], F32, tag="fc1")
                for m in range(8):
                    for kt in range(16):
                        nc.tensor.matmul(ps5[:, m:m + 1],
                                         f1[:, kt * 1024 + m * 128:kt * 1024 + (m + 1) * 128],
                                         v[:, kt:kt + 1],
                                         start=(kt == 0), stop=(kt == 15))
                y1s = c4tmp.tile([128, 8], F32, tag="y1a")
                nc.vector.tensor_tensor(y1s[:], ps5[:], fb1[:], aadd)
                y1b = c4tmp.tile([128, 8], BF16, tag="y1b")
                nc.vector.tensor_scalar_max(y1b[:], y1s[:], 0.0)
                # fc2
                ps6 = c4ps.tile([29, 1], F32, tag="fc2")
                for kt in range(8):
                    nc.tensor.matmul(ps6[:], f2[:, kt * 29:(kt + 1) * 29],
                                     y1b[:, kt:kt + 1],
                                     start=(kt == 0), stop=(kt == 7))
                yout = c4tmp.tile([29, 1], F32, tag="yo")
                nc.vector.tensor_tensor(yout[:], ps6[:], fb2[:], aadd)
                nc.sync.dma_start(AP(y_d, 0, [[1, 29], [1, 1]]), yout[:])
            h4p.release()
            fcp.release()
            h3pool.release()
    nc.compile()
    return nc


def _prep(inputs):
    x = np.asarray(inputs["x"], np.float32)
    sigma = np.asarray(inputs["sigma"], np.float32)
    # Gaussian on the 48-corner: G48[e][i, a'] for cube coords a = a'+22
    coords = np.arange(S, dtype=np.float32) + C0 - 35.0   # a - 35
    idx = np.arange(S, dtype=np.float32)                  # i
    d2 = (coords[None, :] - idx[:, None]) ** 2            # [i, a']
    gt_dev = np.zeros((S, 6 * S), np.float32)
    for e in range(6):
        gt_dev[:, e * S:(e + 1) * S] = np.exp(-d2 / (2.0 * sigma[e] ** 2))

    w1 = np.asarray(inputs["conv1_w"], np.float32)  # [32,6,3,3,3]
    w1a = np.zeros((108, 32), np.float32)
    w1b = np.zeros((54, 32), np.float32)
    for dzp in range(2):
        rb = (1 - dzp) * 54
        for e in range(6):
            for dy in range(3):
                for dx in range(3):
                    w1a[rb + e * 9 + dy * 3 + dx, :] = w1[:, e, dzp, dy, dx]
    for e in range(6):
        for dy in range(3):
            for dx in range(3):
                w1b[e * 9 + dy * 3 + dx, :] = w1[:, e, 2, dy, dx]

    w2 = np.asarray(inputs["conv2_w"], np.float32)  # [64,32,3,3,3]
    w2_dev = np.zeros((96, 576), np.float32)
    for dz in range(3):
        for c in range(32):
            row = dz * 32 + c
            for t in range(9):
                dy, dx = t // 3, t % 3
                w2_dev[row, t * 64:(t + 1) * 64] = w2[:, c, dz, dy, dx]
    w3 = np.asarray(inputs["conv3_w"], np.float32)  # [128,64,3,3,3]
    w3_dev = np.zeros((64, 27 * 128), np.float32)
    for t in range(27):
        dz, dy, dx = t // 9, (t // 3) % 3, t % 3
        w3_dev[:, t * 128:(t + 1) * 128] = w3[:, :, dz, dy, dx].T
    w4 = np.asarray(inputs["conv4_w"], np.float32)  # [256,128,3,3,3]
    w4_dev = np.zeros((128, 27 * 256), np.float32)
    for t in range(27):
        dz, dy, dx = t // 9, (t // 3) % 3, t % 3
        for mt in range(2):
            w4_dev[:, t * 256 + mt * 128:t * 256 + (mt + 1) * 128] = \
                w4[mt * 128:(mt + 1) * 128, :, dz, dy, dx].T
    fc1w = np.asarray(inputs["fc1_w"], np.float32)  # [1024, 2048]
    f1_dev = np.zeros((128, 16 * 1024), np.float32)
    for kt in range(16):
        mt, vox = kt // 8, kt % 8
        for p in range(128):
            f1_dev[p, kt * 1024:(kt + 1) * 1024] = fc1w[:, (mt * 128 + p) * 8 + vox]
    fc2w = np.asarray(inputs["fc2_w"], np.float32)  # [29, 1024]
    f2_dev = np.zeros((128, 8 * 29), np.float32)
    for kt in range(8):
        f2_dev[:, kt * 29:(kt + 1) * 29] = fc2w[:, kt * 128:(kt + 1) * 128].T

    c1b = np.asarray(inputs["conv1_b"], np.float32)
    c2b = np.asarray(inputs["conv2_b"], np.float32)
    bg2 = np.maximum(c1b, 0.0)                           # H2 background
    s2 = np.asarray(inputs["conv2_w"], np.float32).sum(axis=(2, 3, 4))  # [64,32]
    bg3 = np.maximum(c2b + s2 @ bg2, 0.0)                # H3 background

    bf = lambda a: a.astype(ml_dtypes.bfloat16)
    common = dict(
        gt=bf(gt_dev), w1a=bf(w1a), w1b=bf(w1b), w2=bf(w2_dev), w3=bf(w3_dev),
        w4=bf(w4_dev), f1=bf(f1_dev), f2=bf(f2_dev),
        b1=c1b.reshape(32, 1),
        b2=c2b.reshape(64, 1),
        bg2=bg2.reshape(32, 1),
        bg3=bg3.reshape(64, 1),
        b3=np.asarray(inputs["conv3_b"], np.float32).reshape(128, 1),
        b4=np.asarray(inputs["conv4_b"], np.float32).reshape(2, 128).T.copy(),
        fb1=np.asarray(inputs["fc1_b"], np.float32).reshape(8, 128).T.copy(),
        fb2=np.asarray(inputs["fc2_b"], np.float32).reshape(29, 1),
    )
    in_maps = []
    for b in range(8):
        xb = x[b, :, :S, :S, :S].transpose(1, 0, 2, 3).reshape(S, 6 * SS)
        m = dict(common)
        m["xin"] = bf(xb)
        in_maps.append(m)
    return in_maps


def kernel(**inputs):
    if "nc" not in _CACHE:
        _CACHE["nc"] = _build()
    nc = _CACHE["nc"]
    in_maps = _prep(inputs)
    res = run_bass_kernel_spmd(nc, in_maps, core_ids=list(range(8)))
    out = np.stack([res.results[b]["y"] for b in range(8)], axis=0)
    return out.astype(np.float32)


if __name__ == "__main__":
    pass


# revision 27
# speedup vs baseline: 1.0003x; 1.0003x over previous
import sys
import numpy as np

sys.path.insert(0, "/opt/trn_rl_repo")

import concourse.bass as bass  # noqa: E402
import concourse.tile as tile  # noqa: E402
from concourse import bacc, mybir  # noqa: E402
from concourse.ap import AP  # noqa: E402
from concourse.bass_utils import run_bass_kernel_spmd  # noqa: E402
import ml_dtypes  # noqa: E402

BF16 = mybir.dt.bfloat16
F32 = mybir.dt.float32
FP8 = mybir.dt.float8e4

S = 48                 # corner cube side (abs cube coords [22, 70))
C0 = 22                # corner offset in cube coords
SS = S * S             # 2304
CUBE48 = S * SS        # 110592
V1 = 46                # conv1 computed outputs per axis (abs [22, 68))
P2 = 26                # conv2 padded input side (H2 abs [8, 34))
P2S = P2 * P2          # 676
H2C = 23               # H2 corner side (abs [11, 34))
V2 = 24                # conv2 computed outputs per axis (abs [8, 32))
H3S = 16

_CACHE = {}


def _build():
    nc = bacc.Bacc("TRN2", target_bir_lowering=False, debug=False, num_devices=8)
    xin_d = nc.dram_tensor("xin", [S, 6 * SS], BF16, kind="ExternalInput")
    gt_d = nc.dram_tensor("gt", [S, 6 * S], BF16, kind="ExternalInput")
    w1a_d = nc.dram_tensor("w1a", [108, 32], BF16, kind="ExternalInput")
    w1b_d = nc.dram_tensor("w1b", [54, 32], BF16, kind="ExternalInput")
    w2_d = nc.dram_tensor("w2", [96, 576], BF16, kind="ExternalInput")
    w3_d = nc.dram_tensor("w3", [64, 27 * 128], BF16, kind="ExternalInput")
    w4_d = nc.dram_tensor("w4", [128, 27 * 256], BF16, kind="ExternalInput")
    f1_d = nc.dram_tensor("f1", [128, 16 * 1024], BF16, kind="ExternalInput")
    f2_d = nc.dram_tensor("f2", [128, 8 * 29], BF16, kind="ExternalInput")
    b1_d = nc.dram_tensor("b1", [32, 1], F32, kind="ExternalInput")
    b2_d = nc.dram_tensor("b2", [64, 1], F32, kind="ExternalInput")
    bg2_d = nc.dram_tensor("bg2", [32, 1], F32, kind="ExternalInput")
    bg3_d = nc.dram_tensor("bg3", [64, 1], F32, kind="ExternalInput")
    b3_d = nc.dram_tensor("b3", [128, 1], F32, kind="ExternalInput")
    b4_d = nc.dram_tensor("b4", [128, 2], F32, kind="ExternalInput")
    fb1_d = nc.dram_tensor("fb1", [128, 8], F32, kind="ExternalInput")
    fb2_d = nc.dram_tensor("fb2", [29, 1], F32, kind="ExternalInput")
    y_d = nc.dram_tensor("y", [29], F32, kind="ExternalOutput")
    cube_d = nc.dram_tensor("cube", [6 * CUBE48 + 4096], FP8)

    Relu = mybir.ActivationFunctionType.Relu
    Copy = mybir.ActivationFunctionType.Copy
    amax = mybir.AluOpType.max
    aadd = mybir.AluOpType.add

    with tile.TileContext(nc, pool_alloc_mode="queue") as tc:
        with (
            tc.tile_pool(name="const", bufs=1) as constp,
        ):
            gt = constp.tile([S, 6 * S], BF16)
            w1a = constp.tile([108, 32], BF16)
            w1b = constp.tile([54, 32], BF16)
            w2 = constp.tile([96, 576], BF16)
            b1 = constp.tile([32, 1], F32)
            b2 = constp.tile([64, 1], F32)
            bg2 = constp.tile([32, 1], F32)
            bg3 = constp.tile([64, 1], F32)
            b3 = constp.tile([128, 1], F32)
            b4 = constp.tile([128, 2], F32)
            fb1 = constp.tile([128, 8], F32)
            fb2 = constp.tile([29, 1], F32)

            # T1 chunk z-ranges for conv1: output z' in [z0, z1), needs cube
            # slices [z0, z1+2)
            zchunks = [(0, 4), (4, 12), (12, 20), (20, 28), (28, 36), (36, 44), (44, 46)]

            # pools in LIFO lifetime order: h3 (to conv3), T2 (to conv2),
            # T1 (to conv1), xin (blur only)
            h3pool = tc.alloc_tile_pool(name="h3pool", bufs=1)
            H3 = h3pool.tile([64, H3S * H3S * H3S], BF16)
            t2pool = tc.alloc_tile_pool(name="t2pool", bufs=1)
            T2 = t2pool.tile([96, P2 * P2S], BF16)
            t1pool = tc.alloc_tile_pool(name="t1pool", bufs=2)
            MAXCOLS = 10 * SS
            T1 = [t1pool.tile([108, MAXCOLS], FP8, tag="t1", name=f"T1_{i}")
                  for i in range(2)]

            # ---------------- blur (48^3 corner, separable) ----------------
            xinp = tc.alloc_tile_pool(name="xinp", bufs=1)
            xin = xinp.tile([S, 6 * SS], BF16)
            # critical-path loads first, late-needed consts after
            nc.sync.dma_start(gt[:], gt_d[:])
            nc.sync.dma_start(xin[:], xin_d[:])
            for t_, d_ in [(w1a, w1a_d), (w1b, w1b_d), (bg2, bg2_d),
                           (b1, b1_d), (w2, w2_d), (b2, b2_d), (bg3, bg3_d),
                           (b3, b3_d), (b4, b4_d), (fb1, fb1_d), (fb2, fb2_d)]:
                nc.sync.dma_start(t_[:], d_[:])
            xr = xin[:].rearrange("p (e j k) -> p e j k", e=6, j=S, k=S)

            with (
                tc.tile_pool(name="t12", bufs=6) as t12p,
                tc.tile_pool(name="bpsA", bufs=5, space="PSUM") as bpsA,
                tc.tile_pool(name="bpsC", bufs=2, space="PSUM") as bpsC,
                tc.tile_pool(name="cstp", bufs=2) as cstp,
            ):
                # k/a groups for psum staging in stages A and B
                groups = [(0, 10), (10, 10), (20, 10), (30, 10), (40, 8)]
                t1s, t2s = {}, {}

                def stageA(e):
                    ge = gt[:, e * S:(e + 1) * S]
                    t1 = t12p.tile([S, SS], BF16, tag="t", name=f"t1_{e}")
                    t1s[e] = t1
                    for gi, (k0, nk) in enumerate(groups):
                        ps = bpsA.tile([S, 480], F32, tag="ps")
                        for s in range(nk):
                            k = k0 + s
                            nc.tensor.matmul(ps[:, s * S:(s + 1) * S],
                                             xr[:, e, :, k], ge)
                        if gi % 2 == 0:
                            nc.scalar.activation(
                                t1[:, k0 * S:(k0 + nk) * S], ps[:, :nk * S], Copy)
                        else:
                            nc.vector.tensor_copy(
                                t1[:, k0 * S:(k0 + nk) * S], ps[:, :nk * S])

                def stageB(e):
                    ge = gt[:, e * S:(e + 1) * S]
                    t1r = t1s[e][:].rearrange("p (k a) -> p k a", k=S)
                    t2 = t12p.tile([S, SS], BF16, tag="t", name=f"t2_{e}")
                    t2s[e] = t2
                    for gi, (a0, na) in enumerate(groups):
                        ps = bpsA.tile([S, 480], F32, tag="ps")
                        for s in range(na):
                            a = a0 + s
                            nc.tensor.matmul(ps[:, s * S:(s + 1) * S],
                                             t1r[:, :, a], ge)
                        if gi % 2 == 0:
                            nc.scalar.activation(
                                t2[:, a0 * S:(a0 + na) * S], ps[:, :na * S], Copy)
                        else:
                            nc.vector.tensor_copy(
                                t2[:, a0 * S:(a0 + na) * S], ps[:, :na * S])

                def stageC(e):
                    # contract k -> cube chunks [(a,p) 128-chunks, q]
                    ge = gt[:, e * S:(e + 1) * S]
                    t2 = t2s[e]
                    cst = cstp.tile([128, 18 * S], FP8, tag="cst", name=f"cst_{e}")
                    for gi, (c0, ncn) in enumerate([(0, 10), (10, 8)]):
                        ps = bpsC.tile([128, 480], F32, tag="psc")
                        for s in range(ncn):
                            c = c0 + s
                            nc.tensor.matmul(ps[:, s * S:(s + 1) * S],
                                             t2[:, c * 128:(c + 1) * 128], ge)
                        if gi % 2 == 0:
                            nc.scalar.activation(
                                cst[:, c0 * S:(c0 + ncn) * S], ps[:, :ncn * S], Copy)
                        else:
                            nc.vector.tensor_copy(
                                cst[:, c0 * S:(c0 + ncn) * S], ps[:, :ncn * S])
                    cr = cst[:].rearrange("p (c q) -> p c q", c=18)
                    dst = AP(cube_d, e * CUBE48, [[S, 128], [128 * S, 18], [1, S]])
                    nc.sync.dma_start(dst, cr[:])
                    # T1 chunk 0 rows for this element (overlap with blur)
                    # rows [0:54) hold the dz'=1 slices so mm2's rhs starts at
                    # partition 0 (hw requires base partition 0/32/64)
                    z0, z1 = zchunks[0]
                    cols = (z1 + 2 - z0) * SS
                    for dzp in range(2):
                        rb = (1 - dzp) * 54
                        src = AP(cube_d, e * CUBE48 + dzp * SS + z0 * SS,
                                 [[S, 3], [1, 3], [1, cols]])
                        nc.sync.dma_start(
                            T1[0][rb + e * 9:rb + (e + 1) * 9, :cols], src)

                # software-pipelined across e: A(e+1)/B(e) overlap copy drains
                # so the PE stays continuously fed (p-state ramp)
                for step in range(8):
                    if step < 6:
                        stageA(step)
                    if 1 <= step <= 6:
                        stageB(step - 1)
                    if step >= 2:
                        stageC(step - 2)
            xinp.release()

            # ---------------- conv1 (+pool) ----------------
            # T2 holds conv2's z-replica tall tile; rows [0:32] double as H2P
            # (padded H2: bg2 halo + pooled conv1 corner).
            H2P = T2[0:32, :]

            # background fills (independent of conv1 compute)
            # H2P halo: z-planes [0,3), then y<3 rows for z>=3, then x<3 cols
            nc.gpsimd.memset(H2P[:, 0:3 * P2S], 0.0)
            hz = H2P[:, 3 * P2S:].rearrange("p (z y x) -> p z y x", z=P2 - 3, y=P2)
            nc.gpsimd.memset(hz[:, :, 0:3, :], 0.0)
            nc.gpsimd.memset(hz[:, :, 3:, 0:3], 0.0)
            nc.scalar.activation(H2P[:, 0:3 * P2S], H2P[:, 0:3 * P2S],
                                 Relu, bias=bg2[:])
            nc.scalar.activation(hz[:, :, 0:3, :], hz[:, :, 0:3, :],
                                 Relu, bias=bg2[:])
            nc.scalar.activation(hz[:, :, 3:, 0:3], hz[:, :, 3:, 0:3],
                                 Relu, bias=bg2[:])
            # H3 background fill
            nc.gpsimd.memset(H3[:], 0.0)
            nc.scalar.activation(H3[:], H3[:], Relu, bias=bg3[:])

            h2cr = H2P[:].rearrange("p (z y x) -> p z y x", z=P2, y=P2)

            with (
                tc.tile_pool(name="c1ps", bufs=4, space="PSUM") as c1ps,
                tc.tile_pool(name="c1xt", bufs=6) as c1xt,
                tc.tile_pool(name="c1yt", bufs=3) as c1yt,
            ):
                yts = {}
                copied_half1 = False
                for ci, (z0, z1) in enumerate(zchunks):
                    cols = (z1 + 2 - z0) * SS
                    T1c = T1[ci % 2]
                    if ci > 0:
                        for dzp in range(2):
                            rb = (1 - dzp) * 54
                            for e in range(6):
                                src = AP(cube_d,
                                         e * CUBE48 + dzp * SS + z0 * SS,
                                         [[S, 3], [1, 3], [1, cols]])
                                nc.sync.dma_start(
                                    T1c[rb + e * 9:rb + (e + 1) * 9, :cols],
                                    src)
                    for zp in range(z0, z1):
                        zl = (zp - z0) * SS
                        # three y-thirds: psum [32,768] = 2 banks, 4-deep
                        for hi, (y0, ny) in enumerate([(0, 16), (16, 16), (32, 14)]):
                            ncols = ny * S
                            base = zl + y0 * S
                            ps = c1ps.tile([32, 768], F32, tag="ps")
                            for co in range(0, ncols, 512):
                                cw = min(512, ncols - co)
                                nc.tensor.matmul(
                                    ps[:, co:co + cw],
                                    w1a[:], T1c[0:108, base + co:base + co + cw],
                                    start=True, stop=False)
                                nc.tensor.matmul(
                                    ps[:, co:co + cw],
                                    w1b[:], T1c[0:54, base + co + SS:base + co + SS + cw],
                                    start=False, stop=True)
                            # x-pair max: Act drains odd cols to SBUF, DVE
                            # maxes psum-even against sbuf-odd (walrus only
                            # allows one PSUM operand per DVE op)
                            pp = ps[:, 0:ncols].rearrange(
                                "p (y x) -> p y x", y=ny)[:, :, 0:46].rearrange(
                                "p y (xp two) -> p y xp two", two=2)
                            xo = c1xt.tile([32, 368], BF16, tag="xo")
                            xor_ = xo[:, 0:ny * 23].rearrange(
                                "p (y x) -> p y x", y=ny)
                            nc.scalar.activation(xor_[:], pp[:, :, :, 1], Copy)
                            xt = c1xt.tile([32, 368], BF16, tag="xt")
                            xtr = xt[:, 0:ny * 23].rearrange(
                                "p (y x) -> p y x", y=ny)
                            nc.vector.tensor_tensor(
                                xtr[:], pp[:, :, :, 0], xor_[:], amax)
                            # y-pair max into yt slice (pair ranges per third)
                            if hi == 0:
                                yt = c1yt.tile([32, 529], BF16, tag="yt")
                                yts[zp] = yt
                            else:
                                yt = yts[zp]
                            nyp = ny // 2
                            ytr = yt[:, y0 // 2 * 23:(y0 // 2 + nyp) * 23].rearrange(
                                "p (y x) -> p y x", y=nyp)
                            x2 = xt[:, 0:ny * 23].rearrange(
                                "p (y two x) -> p y two x", y=nyp, two=2)
                            nc.vector.tensor_tensor(
                                ytr[:], x2[:, :, 0, :], x2[:, :, 1, :], amax)
                        # z-pair: finish H2 corner slice m = zp//2
                        if zp % 2 == 1:
                            m = zp // 2
                            dst = h2cr[:, 3 + m, 3:, 3:]
                            nc.vector.tensor_tensor(
                                dst, yts[zp - 1][:], yts[zp][:], amax)
                            nc.scalar.activation(dst, dst, Relu, bias=b1[:])
                            del yts[zp - 1], yts[zp]
                        # mid-conv1: copy first z-half of T2 lanes (needs
                        # H2P cols z < 13, i.e. corner m <= 9 plus halo)
                        if zp == 19 and not copied_half1:
                            copied_half1 = True
                            nch = 13 * P2S
                            for lz in (1, 2):
                                nc.sync.dma_start(
                                    T2[32 * lz:32 * (lz + 1), 0:nch - lz * P2S],
                                    T2[0:32, lz * P2S:nch])
                # remaining T2 lane cols
                for lz in (1, 2):
                    nch = 13 * P2S
                    rest = P2 * P2S - lz * P2S
                    nc.sync.dma_start(
                        T2[32 * lz:32 * (lz + 1), nch - lz * P2S:rest],
                        T2[0:32, nch:P2 * P2S])

            t1pool.release()

            # ---------------- conv2 (+pool) ----------------
            h3r16 = H3[:].rearrange("p (z y x) -> p z y x", z=H3S, y=H3S)
            with (
                tc.tile_pool(name="c2ps", bufs=4, space="PSUM") as c2ps,
                tc.tile_pool(name="c2tmp", bufs=6) as c2tmp,
            ):
                y2ts = {}
                for zp in range(V2):
                    base = zp * P2S
                    ps = c2ps.tile([64, 624], F32, tag="ps")
                    for co, cw in [(0, 512), (512, 112)]:
                        for t in range(9):
                            dy, dx = t // 3, t % 3
                            off = base + dy * P2 + dx + co
                            nc.tensor.matmul(
                                ps[:, co:co + cw],
                                w2[:, t * 64:(t + 1) * 64],
                                T2[0:96, off:off + cw],
                                start=(t == 0), stop=(t == 8))
                    # pool: x-pair then y-pair (valid 24x24 of 26-stride)
                    pp = ps[:].rearrange("p (y x) -> p y x", y=24)[
                        :, :, 0:24].rearrange("p y (xp two) -> p y xp two", two=2)
                    xo = c2tmp.tile([64, 288], BF16, tag="xo")
                    xor_ = xo[:].rearrange("p (y x) -> p y x", y=24)
                    nc.scalar.activation(xor_[:], pp[:, :, :, 1], Copy)
                    xt = c2tmp.tile([64, 288], BF16, tag="xt")
                    xtr = xt[:].rearrange("p (y x) -> p y x", y=24)
                    nc.vector.tensor_tensor(
                        xtr[:], pp[:, :, :, 0], xor_[:], amax)
                    yt = c2tmp.tile([64, 144], BF16, tag="yt")
                    ytr = yt[:].rearrange("p (y x) -> p y x", y=12)
                    x2 = xt[:].rearrange("p (y two x) -> p y two x", y=12, two=2)
                    nc.vector.tensor_tensor(
                        ytr[:], x2[:, :, 0, :], x2[:, :, 1, :], amax)
                    y2ts[zp] = yt
                    if zp % 2 == 1:
                        m = zp // 2
                        dst = h3r16[:, 4 + m, 4:, 4:]
                        nc.vector.tensor_tensor(
                            dst, y2ts[zp - 1][:], y2ts[zp][:], amax)
                        nc.scalar.activation(dst, dst, Relu, bias=b2[:])
                        del y2ts[zp - 1], y2ts[zp]
            t2pool.release()
            # conv3/conv4/fc weights (loads overlap conv3 compute)
            fcp = tc.alloc_tile_pool(name="fcp", bufs=1)
            w3 = fcp.tile([64, 27 * 128], BF16)
            nc.sync.dma_start(w3[:], w3_d[:])
            w4 = fcp.tile([128, 27 * 256], BF16)
            nc.sync.dma_start(w4[:], w4_d[:])
            f1 = fcp.tile([128, 16 * 1024], BF16)
            nc.sync.dma_start(f1[:], f1_d[:])
            f2 = fcp.tile([128, 8 * 29], BF16)
            nc.sync.dma_start(f2[:], f2_d[:])

            h4p = tc.alloc_tile_pool(name="h4p", bufs=1)
            H4 = h4p.tile([128, 343], BF16)
            # ---------------- conv3 (baseline) ----------------
            with (
                tc.tile_pool(name="c3ps", bufs=8, space="PSUM") as c3ps,
                tc.tile_pool(name="c3tmp", bufs=16) as c3tmp,
            ):
                h3r = H3[:].rearrange("p (z y x) -> p z y x", z=16, y=16)
                zts = {}
                for half in range(2):
                    pss = []
                    for zi7 in range(7):
                        pszz = c3ps.tile([128, 196], F32, tag="ps")
                        pss.append(pszz)
                    for t in range(27):
                        dz, dy, dx = t // 9, (t // 3) % 3, t % 3
                        for zi in range(7):
                            z = half * 7 + zi
                            rhs = h3r[:, z + dz, dy:dy + 14, dx:dx + 14]
                            nc.tensor.matmul(pss[zi][:], w3[:, t * 128:(t + 1) * 128],
                                             rhs, start=(t == 0), stop=(t == 26))
                    for zi in range(7):
                        z = half * 7 + zi
                        ps = pss[zi]
                        pr = ps[:].rearrange("p (y xp two) -> p y xp two", y=14, two=2)
                        xt = c3tmp.tile([128, 98], F32, tag="xt")
                        xtr = xt[:].rearrange("p (y x) -> p y x", y=14)
                        nc.vector.tensor_reduce(xtr[:], pr[:], mybir.AxisListType.X, amax)
                        yt = c3tmp.tile([128, 49], F32, tag="yt")
                        ytr = yt[:].rearrange("p (y x) -> p y x", y=7)
                        xr2 = xt[:].rearrange("p (yp two x) -> p yp two x", yp=7, two=2)
                        nc.vector.tensor_tensor(ytr[:], xr2[:, :, 0, :], xr2[:, :, 1, :], amax)
                        zts[z] = yt
                for zq in range(7):
                    zt = c3tmp.tile([128, 49], F32, tag="zt")
                    nc.vector.tensor_tensor(zt[:], zts[2 * zq][:], zts[2 * zq + 1][:], amax)
                    nc.scalar.activation(H4[:, zq * 49:(zq + 1) * 49], zt[:],
                                         Relu, bias=b3[:])

            # ---------------- conv4 + fc (baseline) ----------------
            with (
                tc.tile_pool(name="c4ps", bufs=2, space="PSUM") as c4ps,
                tc.tile_pool(name="c4tmp", bufs=8) as c4tmp,
            ):
                h4r = H4[:].rearrange("p (z y x) -> p z y x", z=7, y=7)
                v = c4tmp.tile([128, 16], BF16, tag="v")
                for mt in range(2):
                    ps = c4ps.tile([128, 125], F32, tag="ps")
                    for t in range(27):
                        dz, dy, dx = t // 9, (t // 3) % 3, t % 3
                        rhs = h4r[:, dz:dz + 5, dy:dy + 5, dx:dx + 5]
                        nc.tensor.matmul(ps[:], w4[:, t * 256 + mt * 128:t * 256 + (mt + 1) * 128],
                                         rhs, start=(t == 0), stop=(t == 26))
                    pr0 = ps[:].rearrange("p (z y x) -> p z y x", z=5, y=5)
                    pr = pr0[:, :, :, 0:4].rearrange("p z y (xp two) -> p (z y) xp two", two=2)
                    xt = c4tmp.tile([128, 50], F32, tag="xt")
                    xtr = xt[:].rearrange("p (zy x) -> p zy x", x=2)
                    nc.vector.tensor_reduce(xtr[:], pr[:], mybir.AxisListType.X, amax)
                    x20 = xt[:].rearrange("p (z y x) -> p z y x", z=5, y=5)
                    x2 = x20[:, :, 0:4, :].rearrange("p z (yp two) x -> p z yp two x", two=2)
                    yt = c4tmp.tile([128, 20], F32, tag="yt")
                    ytr = yt[:].rearrange("p (z y x) -> p z y x", z=5, y=2)
                    nc.vector.tensor_tensor(ytr[:], x2[:, :, :, 0, :], x2[:, :, :, 1, :], amax)
                    y2r0 = yt[:].rearrange("p (z yx) -> p z yx", z=5)
                    y2r = y2r0[:, 0:4, :].rearrange("p (zp two) yx -> p zp two yx", two=2)
                    zt = c4tmp.tile([128, 8], F32, tag="zt")
                    ztr = zt[:].rearrange("p (z yx) -> p z yx", z=2)
                    nc.vector.tensor_tensor(ztr[:], y2r[:, :, 0, :], y2r[:, :, 1, :], amax)
                    nc.scalar.activation(v[:, mt * 8:(mt + 1) * 8], zt[:],
                                         Relu, bias=b4[:, mt:mt + 1])
                # fc1
                ps5 = c4ps.tile([128, 8], F32, tag="fc1")
                for m in range(8):
                    for kt in range(16):
                        nc.tensor.matmul(ps5[:, m:m + 1],
                                         f1[:, kt * 1024 + m * 128:kt * 1024 + (m + 1) * 128],
                                         v[:, kt:kt + 1],
                                         start=(kt == 0), stop=(kt == 15))
                y1s = c4tmp.tile([128, 8], F32, tag="y1a")
                nc.vector.tensor_tensor(y1s[:], ps5[:], fb1[:], aadd)
                y1b = c4tmp.tile([128, 8], BF16, tag="y1b")
                nc.vector.tensor_scalar_max(y1b[:], y1s[:], 0.0)
                # fc2
                ps6 = c4ps.tile([29, 1], F32, tag="fc2")
                for kt in range(8):
                    nc.tensor.matmul(ps6[:], f2[:, kt * 29:(kt + 1) * 29],
                                     y1b[:, kt:kt + 1],
                                     start=(kt == 0), stop=(kt == 7))
                yout = c4tmp.tile([29, 1], F32, tag="yo")
                nc.vector.tensor_tensor(yout[:], ps6[:], fb2[:], aadd)
                nc.sync.dma_start(AP(y_d, 0, [[1, 29], [1, 1]]), yout[:])
            h4p.release()
            fcp.release()
            h3pool.release()
    nc.compile()
    return nc


def _prep(inputs):
    x = np.asarray(inputs["x"], np.float32)
    sigma = np.asarray(inputs["sigma"], np.float32)
    # Gaussian on the 48-corner: G48[e][i, a'] for cube coords a = a'+22
    coords = np.arange(S, dtype=np.float32) + C0 - 35.0   # a - 35
    idx = np.arange(S, dtype=np.float32)                  # i
    d2 = (coords[None, :] - idx[:, None]) ** 2            # [i, a']
    gt_dev = np.zeros((S, 6 * S), np.float32)
    for e in range(6):
        gt_dev[:, e * S:(e + 1) * S] = np.exp(-d2 / (2.0 * sigma[e] ** 2))

    w1 = np.asarray(inputs["conv1_w"], np.float32)  # [32,6,3,3,3]
    w1a = np.zeros((108, 32), np.float32)
    w1b = np.zeros((54, 32), np.float32)
    for dzp in range(2):
        rb = (1 - dzp) * 54
        for e in range(6):
            for dy in range(3):
                for dx in range(3):
                    w1a[rb + e * 9 + dy * 3 + dx, :] = w1[:, e, dzp, dy, dx]
    for e in range(6):
        for dy in range(3):
            for dx in range(3):
                w1b[e * 9 + dy * 3 + dx, :] = w1[:, e, 2, dy, dx]

    w2 = np.asarray(inputs["conv2_w"], np.float32)  # [64,32,3,3,3]
    w2_dev = np.zeros((96, 576), np.float32)
    for dz in range(3):
        for c in range(32):
            row = dz * 32 + c
            for t in range(9):
                dy, dx = t // 3, t % 3
                w2_dev[row, t * 64:(t + 1) * 64] = w2[:, c, dz, dy, dx]
    w3 = np.asarray(inputs["conv3_w"], np.float32)  # [128,64,3,3,3]
    w3_dev = np.zeros((64, 27 * 128), np.float32)
    for t in range(27):
        dz, dy, dx = t // 9, (t // 3) % 3, t % 3
        w3_dev[:, t * 128:(t + 1) * 128] = w3[:, :, dz, dy, dx].T
    w4 = np.asarray(inputs["conv4_w"], np.float32)  # [256,128,3,3,3]
    w4_dev = np.zeros((128, 27 * 256), np.float32)
    for t in range(27):
        dz, dy, dx = t // 9, (t // 3) % 3, t % 3
        for mt in range(2):
            w4_dev[:, t * 256 + mt * 128:t * 256 + (mt + 1) * 128] = \
                w4[mt * 128:(mt + 1) * 128, :, dz, dy, dx].T
    fc1w = np.asarray(inputs["fc1_w"], np.float32)  # [1024, 2048]
    f1_dev = np.zeros((128, 16 * 1024), np.float32)
    for kt in range(16):
        mt, vox = kt // 8, kt % 8
        for p in range(128):
            f1_dev[p, kt * 1024:(kt + 1) * 1024] = fc1w[:, (mt * 128 + p) * 8 + vox]
    fc2w = np.asarray(inputs["fc2_w"], np.float32)  # [29, 1024]
    f2_dev = np.zeros((128, 8 * 29), np.float32)
    for kt in range(8):
        f2_dev[:, kt * 29:(kt + 1) * 29] = fc2w[:, kt * 128:(kt + 1) * 128].T

    c1b = np.asarray(inputs["conv1_b"], np.float32)
    c2b = np.asarray(inputs["conv2_b"], np.float32)
    bg2 = np.maximum(c1b, 0.0)                           # H2 background
    s2 = np.asarray(inputs["conv2_w"], np.float32).sum(axis=(2, 3, 4))  # [64,32]
    bg3 = np.maximum(c2b + s2 @ bg2, 0.0)                # H3 background

    bf = lambda a: a.astype(ml_dtypes.bfloat16)
    common = dict(
        gt=bf(gt_dev), w1a=bf(w1a), w1b=bf(w1b), w2=bf(w2_dev), w3=bf(w3_dev),
        w4=bf(w4_dev), f1=bf(f1_dev), f2=bf(f2_dev),
        b1=c1b.reshape(32, 1),
        b2=c2b.reshape(64, 1),
        bg2=bg2.reshape(32, 1),
        bg3=bg3.reshape(64, 1),
        b3=np.asarray(inputs["conv3_b"], np.float32).reshape(128, 1),
        b4=np.asarray(inputs["conv4_b"], np.float32).reshape(2, 128).T.copy(),
        fb1=np.asarray(inputs["fc1_b"], np.float32).reshape(8, 128).T.copy(),
        fb2=np.asarray(inputs["fc2_b"], np.float32).reshape(29, 1),
    )
    in_maps = []
    for b in range(8):
        xb = x[b, :, :S, :S, :S].transpose(1, 0, 2, 3).reshape(S, 6 * SS)
        m = dict(common)
        m["xin"] = bf(xb)
        in_maps.append(m)
    return in_maps


def kernel(**inputs):
    if "nc" not in _CACHE:
        _CACHE["nc"] = _build()
    nc = _CACHE["nc"]
    in_maps = _prep(inputs)
    res = run_bass_kernel_spmd(nc, in_maps, core_ids=list(range(8)))
    out = np.stack([res.results[b]["y"] for b in range(8)], axis=0)
    return out.astype(np.float32)


if __name__ == "__main__":
    pass


# revision 28
# speedup vs baseline: 1.0272x; 1.0269x over previous
import sys
import numpy as np

sys.path.insert(0, "/opt/trn_rl_repo")

import concourse.bass as bass  # noqa: E402
import concourse.tile as tile  # noqa: E402
from concourse import bacc, mybir  # noqa: E402
from concourse.ap import AP  # noqa: E402
from concourse.bass_utils import run_bass_kernel_spmd  # noqa: E402
import ml_dtypes  # noqa: E402

BF16 = mybir.dt.bfloat16
F32 = mybir.dt.float32
FP8 = mybir.dt.float8e4

S = 48                 # corner cube side (abs cube coords [22, 70))
C0 = 22                # corner offset in cube coords
SS = S * S             # 2304
CUBE48 = S * SS        # 110592
V1 = 46                # conv1 computed outputs per axis (abs [22, 68))
P2 = 26                # conv2 padded input side (H2 abs [8, 34))
P2S = P2 * P2          # 676
H2C = 23               # H2 corner side (abs [11, 34))
V2 = 24                # conv2 computed outputs per axis (abs [8, 32))
H3S = 16

_CACHE = {}


def _build():
    nc = bacc.Bacc("TRN2", target_bir_lowering=False, debug=False, num_devices=8)
    xin_d = nc.dram_tensor("xin", [S, 6 * SS], BF16, kind="ExternalInput")
    gt_d = nc.dram_tensor("gt", [S, 6 * S], BF16, kind="ExternalInput")
    w1a_d = nc.dram_tensor("w1a", [108, 32], BF16, kind="ExternalInput")
    w1b_d = nc.dram_tensor("w1b", [54, 32], BF16, kind="ExternalInput")
    w2_d = nc.dram_tensor("w2", [96, 576], BF16, kind="ExternalInput")
    w3_d = nc.dram_tensor("w3", [64, 27 * 128], BF16, kind="ExternalInput")
    w4_d = nc.dram_tensor("w4", [128, 27 * 256], BF16, kind="ExternalInput")
    f1_d = nc.dram_tensor("f1", [128, 16 * 1024], BF16, kind="ExternalInput")
    f2_d = nc.dram_tensor("f2", [128, 8 * 29], BF16, kind="ExternalInput")
    b1_d = nc.dram_tensor("b1", [32, 1], F32, kind="ExternalInput")
    b2_d = nc.dram_tensor("b2", [64, 1], F32, kind="ExternalInput")
    bg2_d = nc.dram_tensor("bg2", [32, 1], F32, kind="ExternalInput")
    bg3_d = nc.dram_tensor("bg3", [64, 1], F32, kind="ExternalInput")
    b3_d = nc.dram_tensor("b3", [128, 1], F32, kind="ExternalInput")
    b4_d = nc.dram_tensor("b4", [128, 2], F32, kind="ExternalInput")
    fb1_d = nc.dram_tensor("fb1", [128, 8], F32, kind="ExternalInput")
    fb2_d = nc.dram_tensor("fb2", [29, 1], F32, kind="ExternalInput")
    y_d = nc.dram_tensor("y", [29], F32, kind="ExternalOutput")
    cube_d = nc.dram_tensor("cube", [6 * CUBE48 + 4096], FP8)

    Relu = mybir.ActivationFunctionType.Relu
    Copy = mybir.ActivationFunctionType.Copy
    amax = mybir.AluOpType.max
    aadd = mybir.AluOpType.add

    with tile.TileContext(nc, pool_alloc_mode="queue") as tc:
        with (
            tc.tile_pool(name="const", bufs=1) as constp,
        ):
            gt = constp.tile([S, 6 * S], BF16)
            w1a = constp.tile([108, 32], BF16)
            w1b = constp.tile([54, 32], BF16)
            w2 = constp.tile([96, 576], BF16)
            b1 = constp.tile([32, 1], F32)
            b2 = constp.tile([64, 1], F32)
            bg2 = constp.tile([32, 1], F32)
            bg3 = constp.tile([64, 1], F32)
            b3 = constp.tile([128, 1], F32)
            b4 = constp.tile([128, 2], F32)
            fb1 = constp.tile([128, 8], F32)
            fb2 = constp.tile([29, 1], F32)

            # T1 chunk z-ranges for conv1: output z' in [z0, z1), needs cube
            # slices [z0, z1+2)
            zchunks = [(0, 4), (4, 12), (12, 20), (20, 28), (28, 36), (36, 44), (44, 46)]

            # pools in LIFO lifetime order: h3 (to conv3), T2 (to conv2),
            # T1 (to conv1), xin (blur only)
            h3pool = tc.alloc_tile_pool(name="h3pool", bufs=1)
            H3 = h3pool.tile([64, H3S * H3S * H3S], BF16)
            t2pool = tc.alloc_tile_pool(name="t2pool", bufs=1)
            T2 = t2pool.tile([96, P2 * P2S], BF16)
            t1pool = tc.alloc_tile_pool(name="t1pool", bufs=2)
            MAXCOLS = 10 * SS
            T1 = [t1pool.tile([108, MAXCOLS], FP8, tag="t1", name=f"T1_{i}")
                  for i in range(2)]

            # ---------------- blur (48^3 corner, separable) ----------------
            xinp = tc.alloc_tile_pool(name="xinp", bufs=1)
            xin = xinp.tile([S, 6 * SS], BF16)
            # critical-path loads first, late-needed consts after
            nc.sync.dma_start(gt[:], gt_d[:])
            for e_ in range(6):
                nc.sync.dma_start(xin[:, e_ * SS:(e_ + 1) * SS],
                                  xin_d[:, e_ * SS:(e_ + 1) * SS])
            for t_, d_ in [(w1a, w1a_d), (w1b, w1b_d), (bg2, bg2_d),
                           (b1, b1_d), (w2, w2_d), (b2, b2_d), (bg3, bg3_d),
                           (b3, b3_d), (b4, b4_d), (fb1, fb1_d), (fb2, fb2_d)]:
                nc.sync.dma_start(t_[:], d_[:])
            xr = xin[:].rearrange("p (e j k) -> p e j k", e=6, j=S, k=S)

            with (
                tc.tile_pool(name="t12", bufs=6) as t12p,
                tc.tile_pool(name="bpsA", bufs=5, space="PSUM") as bpsA,
                tc.tile_pool(name="bpsC", bufs=2, space="PSUM") as bpsC,
                tc.tile_pool(name="cstp", bufs=2) as cstp,
            ):
                # k/a groups for psum staging in stages A and B
                groups = [(0, 10), (10, 10), (20, 10), (30, 10), (40, 8)]
                t1s, t2s = {}, {}

                def stageA(e):
                    ge = gt[:, e * S:(e + 1) * S]
                    t1 = t12p.tile([S, SS], BF16, tag="t", name=f"t1_{e}")
                    t1s[e] = t1
                    for gi, (k0, nk) in enumerate(groups):
                        ps = bpsA.tile([S, 480], F32, tag="ps")
                        for s in range(nk):
                            k = k0 + s
                            nc.tensor.matmul(ps[:, s * S:(s + 1) * S],
                                             xr[:, e, :, k], ge)
                        if gi % 2 == 0:
                            nc.scalar.activation(
                                t1[:, k0 * S:(k0 + nk) * S], ps[:, :nk * S], Copy)
                        else:
                            nc.vector.tensor_copy(
                                t1[:, k0 * S:(k0 + nk) * S], ps[:, :nk * S])

                def stageB(e):
                    ge = gt[:, e * S:(e + 1) * S]
                    t1r = t1s[e][:].rearrange("p (k a) -> p k a", k=S)
                    t2 = t12p.tile([S, SS], BF16, tag="t", name=f"t2_{e}")
                    t2s[e] = t2
                    for gi, (a0, na) in enumerate(groups):
                        ps = bpsA.tile([S, 480], F32, tag="ps")
                        for s in range(na):
                            a = a0 + s
                            nc.tensor.matmul(ps[:, s * S:(s + 1) * S],
                                             t1r[:, :, a], ge)
                        if gi % 2 == 0:
                            nc.scalar.activation(
                                t2[:, a0 * S:(a0 + na) * S], ps[:, :na * S], Copy)
                        else:
                            nc.vector.tensor_copy(
                                t2[:, a0 * S:(a0 + na) * S], ps[:, :na * S])

                def stageC(e):
                    # contract k -> cube chunks [(a,p) 128-chunks, q]
                    ge = gt[:, e * S:(e + 1) * S]
                    t2 = t2s[e]
                    cst = cstp.tile([128, 18 * S], FP8, tag="cst", name=f"cst_{e}")
                    for gi, (c0, ncn) in enumerate([(0, 10), (10, 8)]):
                        ps = bpsC.tile([128, 480], F32, tag="psc")
                        for s in range(ncn):
                            c = c0 + s
                            nc.tensor.matmul(ps[:, s * S:(s + 1) * S],
                                             t2[:, c * 128:(c + 1) * 128], ge)
                        if gi % 2 == 0:
                            nc.scalar.activation(
                                cst[:, c0 * S:(c0 + ncn) * S], ps[:, :ncn * S], Copy)
                        else:
                            nc.vector.tensor_copy(
                                cst[:, c0 * S:(c0 + ncn) * S], ps[:, :ncn * S])
                    cr = cst[:].rearrange("p (c q) -> p c q", c=18)
                    dst = AP(cube_d, e * CUBE48, [[S, 128], [128 * S, 18], [1, S]])
                    nc.sync.dma_start(dst, cr[:])
                    # T1 chunk 0 rows for this element (overlap with blur)
                    # rows [0:54) hold the dz'=1 slices so mm2's rhs starts at
                    # partition 0 (hw requires base partition 0/32/64)
                    z0, z1 = zchunks[0]
                    cols = (z1 + 2 - z0) * SS
                    for dzp in range(2):
                        rb = (1 - dzp) * 54
                        src = AP(cube_d, e * CUBE48 + dzp * SS + z0 * SS,
                                 [[S, 3], [1, 3], [1, cols]])
                        nc.sync.dma_start(
                            T1[0][rb + e * 9:rb + (e + 1) * 9, :cols], src)

                # software-pipelined across e: A(e+1)/B(e) overlap copy drains
                # so the PE stays continuously fed (p-state ramp)
                for step in range(8):
                    if step < 6:
                        stageA(step)
                    if 1 <= step <= 6:
                        stageB(step - 1)
                    if step >= 2:
                        stageC(step - 2)
            xinp.release()

            # ---------------- conv1 (+pool) ----------------
            # T2 holds conv2's z-replica tall tile; rows [0:32] double as H2P
            # (padded H2: bg2 halo + pooled conv1 corner).
            H2P = T2[0:32, :]

            # background fills (independent of conv1 compute)
            # H2P halo: z-planes [0,3), then y<3 rows for z>=3, then x<3 cols
            nc.gpsimd.memset(H2P[:, 0:3 * P2S], 0.0)
            hz = H2P[:, 3 * P2S:].rearrange("p (z y x) -> p z y x", z=P2 - 3, y=P2)
            nc.gpsimd.memset(hz[:, :, 0:3, :], 0.0)
            nc.gpsimd.memset(hz[:, :, 3:, 0:3], 0.0)
            nc.scalar.activation(H2P[:, 0:3 * P2S], H2P[:, 0:3 * P2S],
                                 Relu, bias=bg2[:])
            nc.scalar.activation(hz[:, :, 0:3, :], hz[:, :, 0:3, :],
                                 Relu, bias=bg2[:])
            nc.scalar.activation(hz[:, :, 3:, 0:3], hz[:, :, 3:, 0:3],
                                 Relu, bias=bg2[:])
            # H3 background fill
            nc.gpsimd.memset(H3[:], 0.0)
            nc.scalar.activation(H3[:], H3[:], Relu, bias=bg3[:])

            h2cr = H2P[:].rearrange("p (z y x) -> p z y x", z=P2, y=P2)

            with (
                tc.tile_pool(name="c1ps", bufs=4, space="PSUM") as c1ps,
                tc.tile_pool(name="c1xt", bufs=6) as c1xt,
                tc.tile_pool(name="c1yt", bufs=3) as c1yt,
            ):
                yts = {}
                copied_half1 = False
                for ci, (z0, z1) in enumerate(zchunks):
                    cols = (z1 + 2 - z0) * SS
                    T1c = T1[ci % 2]
                    if ci > 0:
                        for dzp in range(2):
                            rb = (1 - dzp) * 54
                            for e in range(6):
                                src = AP(cube_d,
                                         e * CUBE48 + dzp * SS + z0 * SS,
                                         [[S, 3], [1, 3], [1, cols]])
                                nc.sync.dma_start(
                                    T1c[rb + e * 9:rb + (e + 1) * 9, :cols],
                                    src)
                    for zp in range(z0, z1):
                        zl = (zp - z0) * SS
                        # three y-thirds: psum [32,768] = 2 banks, 4-deep
                        for hi, (y0, ny) in enumerate([(0, 16), (16, 16), (32, 14)]):
                            ncols = ny * S
                            base = zl + y0 * S
                            ps = c1ps.tile([32, 768], F32, tag="ps")
                            for co in range(0, ncols, 512):
                                cw = min(512, ncols - co)
                                nc.tensor.matmul(
                                    ps[:, co:co + cw],
                                    w1a[:], T1c[0:108, base + co:base + co + cw],
                                    start=True, stop=False)
                                nc.tensor.matmul(
                                    ps[:, co:co + cw],
                                    w1b[:], T1c[0:54, base + co + SS:base + co + SS + cw],
                                    start=False, stop=True)
                            # x-pair max: Act drains odd cols to SBUF, DVE
                            # maxes psum-even against sbuf-odd (walrus only
                            # allows one PSUM operand per DVE op)
                            pp = ps[:, 0:ncols].rearrange(
                                "p (y x) -> p y x", y=ny)[:, :, 0:46].rearrange(
                                "p y (xp two) -> p y xp two", two=2)
                            xo = c1xt.tile([32, 368], BF16, tag="xo")
                            xor_ = xo[:, 0:ny * 23].rearrange(
                                "p (y x) -> p y x", y=ny)
                            nc.scalar.activation(xor_[:], pp[:, :, :, 1], Copy)
                            if hi == 0:
                                xtz = c1xt.tile([32, 1058], BF16, tag="xt")
                            xtr = xtz[:, y0 * 23:(y0 + ny) * 23].rearrange(
                                "p (y x) -> p y x", y=ny)
                            nc.vector.tensor_tensor(
                                xtr[:], pp[:, :, :, 0], xor_[:], amax)
                        # single y-pair over the whole z-slice
                        yt = c1yt.tile([32, 529], BF16, tag="yt")
                        yts[zp] = yt
                        ytr = yt[:].rearrange("p (y x) -> p y x", y=23)
                        xz = xtz[:].rearrange("p (y two x) -> p y two x",
                                              y=23, two=2)
                        nc.vector.tensor_tensor(
                            ytr[:], xz[:, :, 0, :], xz[:, :, 1, :], amax)
                        # z-pair: finish H2 corner slice m = zp//2
                        if zp % 2 == 1:
                            m = zp // 2
                            dst = h2cr[:, 3 + m, 3:, 3:]
                            nc.vector.tensor_tensor(
                                dst, yts[zp - 1][:], yts[zp][:], amax)
                            nc.scalar.activation(dst, dst, Relu, bias=b1[:])
                            del yts[zp - 1], yts[zp]
                        # mid-conv1: copy first z-half of T2 lanes (needs
                        # H2P cols z < 13, i.e. corner m <= 9 plus halo)
                        if zp == 19 and not copied_half1:
                            copied_half1 = True
                            nch = 13 * P2S
                            for lz in (1, 2):
                                nc.sync.dma_start(
                                    T2[32 * lz:32 * (lz + 1), 0:nch - lz * P2S],
                                    T2[0:32, lz * P2S:nch])
                # remaining T2 lane cols
                for lz in (1, 2):
                    nch = 13 * P2S
                    rest = P2 * P2S - lz * P2S
                    nc.sync.dma_start(
                        T2[32 * lz:32 * (lz + 1), nch - lz * P2S:rest],
                        T2[0:32, nch:P2 * P2S])

            t1pool.release()

            # ---------------- conv2 (+pool) ----------------
            h3r16 = H3[:].rearrange("p (z y x) -> p z y x", z=H3S, y=H3S)
            with (
                tc.tile_pool(name="c2ps", bufs=4, space="PSUM") as c2ps,
                tc.tile_pool(name="c2tmp", bufs=6) as c2tmp,
            ):
                y2ts = {}
                for zp in range(V2):
                    base = zp * P2S
                    ps = c2ps.tile([64, 624], F32, tag="ps")
                    for co, cw in [(0, 512), (512, 112)]:
                        for t in range(9):
                            dy, dx = t // 3, t % 3
                            off = base + dy * P2 + dx + co
                            nc.tensor.matmul(
                                ps[:, co:co + cw],
                                w2[:, t * 64:(t + 1) * 64],
                                T2[0:96, off:off + cw],
                                start=(t == 0), stop=(t == 8))
                    # pool: x-pair then y-pair (valid 24x24 of 26-stride)
                    pp = ps[:].rearrange("p (y x) -> p y x", y=24)[
                        :, :, 0:24].rearrange("p y (xp two) -> p y xp two", two=2)
                    xo = c2tmp.tile([64, 288], BF16, tag="xo")
                    xor_ = xo[:].rearrange("p (y x) -> p y x", y=24)
                    nc.scalar.activation(xor_[:], pp[:, :, :, 1], Copy)
                    xt = c2tmp.tile([64, 288], BF16, tag="xt")
                    xtr = xt[:].rearrange("p (y x) -> p y x", y=24)
                    nc.vector.tensor_tensor(
                        xtr[:], pp[:, :, :, 0], xor_[:], amax)
                    yt = c2tmp.tile([64, 144], BF16, tag="yt")
                    ytr = yt[:].rearrange("p (y x) -> p y x", y=12)
                    x2 = xt[:].rearrange("p (y two x) -> p y two x", y=12, two=2)
                    nc.vector.tensor_tensor(
                        ytr[:], x2[:, :, 0, :], x2[:, :, 1, :], amax)
                    y2ts[zp] = yt
                    if zp % 2 == 1:
                        m = zp // 2
                        dst = h3r16[:, 4 + m, 4:, 4:]
                        nc.vector.tensor_tensor(
                            dst, y2ts[zp - 1][:], y2ts[zp][:], amax)
                        nc.scalar.activation(dst, dst, Relu, bias=b2[:])
                        del y2ts[zp - 1], y2ts[zp]
            t2pool.release()
            # conv3/conv4/fc weights (loads overlap conv3 compute)
            fcp = tc.alloc_tile_pool(name="fcp", bufs=1)
            w3 = fcp.tile([64, 27 * 128], BF16)
            nc.sync.dma_start(w3[:], w3_d[:])
            w4 = fcp.tile([128, 27 * 256], BF16)
            nc.sync.dma_start(w4[:], w4_d[:])
            f1 = fcp.tile([128, 16 * 1024], BF16)
            nc.sync.dma_start(f1[:], f1_d[:])
            f2 = fcp.tile([128, 8 * 29], BF16)
            nc.sync.dma_start(f2[:], f2_d[:])

            h4p = tc.alloc_tile_pool(name="h4p", bufs=1)
            H4 = h4p.tile([128, 343], BF16)
            # ---------------- conv3 (baseline) ----------------
            with (
                tc.tile_pool(name="c3ps", bufs=8, space="PSUM") as c3ps,
                tc.tile_pool(name="c3tmp", bufs=16) as c3tmp,
            ):
                h3r = H3[:].rearrange("p (z y x) -> p z y x", z=16, y=16)
                zts = {}
                for half in range(2):
                    pss = []
                    for zi7 in range(7):
                        pszz = c3ps.tile([128, 196], F32, tag="ps")
                        pss.append(pszz)
                    for t in range(27):
                        dz, dy, dx = t // 9, (t // 3) % 3, t % 3
                        for zi in range(7):
                            z = half * 7 + zi
                            rhs = h3r[:, z + dz, dy:dy + 14, dx:dx + 14]
                            nc.tensor.matmul(pss[zi][:], w3[:, t * 128:(t + 1) * 128],
                                             rhs, start=(t == 0), stop=(t == 26))
                    for zi in range(7):
                        z = half * 7 + zi
                        ps = pss[zi]
                        pr = ps[:].rearrange("p (y xp two) -> p y xp two", y=14, two=2)
                        xt = c3tmp.tile([128, 98], F32, tag="xt")
                        xtr = xt[:].rearrange("p (y x) -> p y x", y=14)
                        nc.vector.tensor_reduce(xtr[:], pr[:], mybir.AxisListType.X, amax)
                        yt = c3tmp.tile([128, 49], F32, tag="yt")
                        ytr = yt[:].rearrange("p (y x) -> p y x", y=7)
                        xr2 = xt[:].rearrange("p (yp two x) -> p yp two x", yp=7, two=2)
                        nc.vector.tensor_tensor(ytr[:], xr2[:, :, 0, :], xr2[:, :, 1, :], amax)
                        zts[z] = yt
                for zq in range(7):
                    zt = c3tmp.tile([128, 49], F32, tag="zt")
                    nc.vector.tensor_tensor(zt[:], zts[2 * zq][:], zts[2 * zq + 1][:], amax)
                    nc.scalar.activation(H4[:, zq * 49:(zq + 1) * 49], zt[:],
                                         Relu, bias=b3[:])

            # ---------------- conv4 + fc (baseline) ----------------
            with (
                tc.tile_pool(name="c4ps", bufs=2, space="PSUM") as c4ps,
                tc.tile_pool(name="c4tmp", bufs=8) as c4tmp,
            ):
                h4r = H4[:].rearrange("p (z y x) -> p z y x", z=7, y=7)
                v = c4tmp.tile([128, 16], BF16, tag="v")
                for mt in range(2):
                    ps = c4ps.tile([128, 125], F32, tag="ps")
                    for t in range(27):
                        dz, dy, dx = t // 9, (t // 3) % 3, t % 3
                        rhs = h4r[:, dz:dz + 5, dy:dy + 5, dx:dx + 5]
                        nc.tensor.matmul(ps[:], w4[:, t * 256 + mt * 128:t * 256 + (mt + 1) * 128],
                                         rhs, start=(t == 0), stop=(t == 26))
                    pr0 = ps[:].rearrange("p (z y x) -> p z y x", z=5, y=5)
                    pr = pr0[:, :, :, 0:4].rearrange("p z y (xp two) -> p (z y) xp two", two=2)
                    xt = c4tmp.tile([128, 50], F32, tag="xt")
                    xtr = xt[:].rearrange("p (zy x) -> p zy x", x=2)
                    nc.vector.tensor_reduce(xtr[:], pr[:], mybir.AxisListType.X, amax)
                    x20 = xt[:].rearrange("p (z y x) -> p z y x", z=5, y=5)
                    x2 = x20[:, :, 0:4, :].rearrange("p z (yp two) x -> p z yp two x", two=2)
                    yt = c4tmp.tile([128, 20], F32, tag="yt")
                    ytr = yt[:].rearrange("p (z y x) -> p z y x", z=5, y=2)
                    nc.vector.tensor_tensor(ytr[:], x2[:, :, :, 0, :], x2[:, :, :, 1, :], amax)
                    y2r0 = yt[:].rearrange("p (z yx) -> p z yx", z=5)
                    y2r = y2r0[:, 0:4, :].rearrange("p (zp two) yx -> p zp two yx", two=2)
                    zt = c4tmp.tile([128, 8], F32, tag="zt")
                    ztr = zt[:].rearrange("p (z yx) -> p z yx", z=2)
                    nc.vector.tensor_tensor(ztr[:], y2r[:, :, 0, :], y2r[:, :, 1, :], amax)
                    nc.scalar.activation(v[:, mt * 8:(mt + 1) * 8], zt[:],
                                         Relu, bias=b4[:, mt:mt + 1])
                # fc1
                ps5 = c4ps.tile([128, 8], F32, tag="fc1")
                for m in range(8):
                    for kt in range(16):
                        nc.tensor.matmul(ps5[:, m:m + 1],
                                         f1[:, kt * 1024 + m * 128:kt * 1024 + (m + 1) * 128],
                                         v[:, kt:kt + 1],
                                         start=(kt == 0), stop=(kt == 15))
                y1s = c4tmp.tile([128, 8], F32, tag="y1a")
                nc.vector.tensor_tensor(y1s[:], ps5[:], fb1[:], aadd)
                y1b = c4tmp.tile([128, 8], BF16, tag="y1b")
                nc.vector.tensor_scalar_max(y1b[:], y1s[:], 0.0)
                # fc2
                ps6 = c4ps.tile([29, 1], F32, tag="fc2")
                for kt in range(8):
                    nc.tensor.matmul(ps6[:], f2[:, kt * 29:(kt + 1) * 29],
                                     y1b[:, kt:kt + 1],
                                     start=(kt == 0), stop=(kt == 7))
                yout = c4tmp.tile([29, 1], F32, tag="yo")
                nc.vector.tensor_tensor(yout[:], ps6[:], fb2[:], aadd)
                nc.sync.dma_start(AP(y_d, 0, [[1, 29], [1, 1]]), yout[:])
            h4p.release()
            fcp.release()
            h3pool.release()
    nc.compile()
    return nc


def _prep(inputs):
    x = np.asarray(inputs["x"], np.float32)
    sigma = np.asarray(inputs["sigma"], np.float32)
    # Gaussian on the 48-corner: G48[e][i, a'] for cube coords a = a'+22
    coords = np.arange(S, dtype=np.float32) + C0 - 35.0   # a - 35
    idx = np.arange(S, dtype=np.float32)                  # i
    d2 = (coords[None, :] - idx[:, None]) ** 2            # [i, a']
    gt_dev = np.zeros((S, 6 * S), np.float32)
    for e in range(6):
        gt_dev[:, e * S:(e + 1) * S] = np.exp(-d2 / (2.0 * sigma[e] ** 2))

    w1 = np.asarray(inputs["conv1_w"], np.float32)  # [32,6,3,3,3]
    w1a = np.zeros((108, 32), np.float32)
    w1b = np.zeros((54, 32), np.float32)
    for dzp in range(2):
        rb = (1 - dzp) * 54
        for e in range(6):
            for dy in range(3):
                for dx in range(3):
                    w1a[rb + e * 9 + dy * 3 + dx, :] = w1[:, e, dzp, dy, dx]
    for e in range(6):
        for dy in range(3):
            for dx in range(3):
                w1b[e * 9 + dy * 3 + dx, :] = w1[:, e, 2, dy, dx]

    w2 = np.asarray(inputs["conv2_w"], np.float32)  # [64,32,3,3,3]
    w2_dev = np.zeros((96, 576), np.float32)
    for dz in range(3):
        for c in range(32):
            row = dz * 32 + c
            for t in range(9):
                dy, dx = t // 3, t % 3
                w2_dev[row, t * 64:(t + 1) * 64] = w2[:, c, dz, dy, dx]
    w3 = np.asarray(inputs["conv3_w"], np.float32)  # [128,64,3,3,3]
    w3_dev = np.zeros((64, 27 * 128), np.float32)
    for t in range(27):
        dz, dy, dx = t // 9, (t // 3) % 3, t % 3
        w3_dev[:, t * 128:(t + 1) * 128] = w3[:, :, dz, dy, dx].T
    w4 = np.asarray(inputs["conv4_w"], np.float32)  # [256,128,3,3,3]
    w4_dev = np.zeros((128, 27 * 256), np.float32)
    for t in range(27):
        dz, dy, dx = t // 9, (t // 3) % 3, t % 3
        for mt in range(2):
            w4_dev[:, t * 256 + mt * 128:t * 256 + (mt + 1) * 128] = \
                w4[mt * 128:(mt + 1) * 128, :, dz, dy, dx].T
    fc1w = np.asarray(inputs["fc1_w"], np.float32)  # [1024, 2048]
    f1_dev = np.zeros((128, 16 * 1024), np.float32)
    for kt in range(16):
        mt, vox = kt // 8, kt % 8
        for p in range(128):
            f1_dev[p, kt * 1024:(kt + 1) * 1024] = fc1w[:, (mt * 128 + p) * 8 + vox]
    fc2w = np.asarray(inputs["fc2_w"], np.float32)  # [29, 1024]
    f2_dev = np.zeros((128, 8 * 29), np.float32)
    for kt in range(8):
        f2_dev[:, kt * 29:(kt + 1) * 29] = fc2w[:, kt * 128:(kt + 1) * 128].T

    c1b = np.asarray(inputs["conv1_b"], np.float32)
    c2b = np.asarray(inputs["conv2_b"], np.float32)
    bg2 = np.maximum(c1b, 0.0)                           # H2 background
    s2 = np.asarray(inputs["conv2_w"], np.float32).sum(axis=(2, 3, 4))  # [64,32]
    bg3 = np.maximum(c2b + s2 @ bg2, 0.0)                # H3 background

    bf = lambda a: a.astype(ml_dtypes.bfloat16)
    common = dict(
        gt=bf(gt_dev), w1a=bf(w1a), w1b=bf(w1b), w2=bf(w2_dev), w3=bf(w3_dev),
        w4=bf(w4_dev), f1=bf(f1_dev), f2=bf(f2_dev),
        b1=c1b.reshape(32, 1),
        b2=c2b.reshape(64, 1),
        bg2=bg2.reshape(32, 1),
        bg3=bg3.reshape(64, 1),
        b3=np.asarray(inputs["conv3_b"], np.float32).reshape(128, 1),
        b4=np.asarray(inputs["conv4_b"], np.float32).reshape(2, 128).T.copy(),
        fb1=np.asarray(inputs["fc1_b"], np.float32).reshape(8, 128).T.copy(),
        fb2=np.asarray(inputs["fc2_b"], np.float32).reshape(29, 1),
    )
    in_maps = []
    for b in range(8):
        xb = x[b, :, :S, :S, :S].transpose(1, 0, 2, 3).reshape(S, 6 * SS)
        m = dict(common)
        m["xin"] = bf(xb)
        in_maps.append(m)
    return in_maps


def kernel(**inputs):
    if "nc" not in _CACHE:
        _CACHE["nc"] = _build()
    nc = _CACHE["nc"]
    in_maps = _prep(inputs)
    res = run_bass_kernel_spmd(nc, in_maps, core_ids=list(range(8)))
    out = np.stack([res.results[b]["y"] for b in range(8)], axis=0)
    return out.astype(np.float32)


if __name__ == "__main__":
    pass
